# revision 1
# baseline (speedup 1.0000x reference)
"""Multi-head attention Trainium2 kernel, 8-core SPMD.

Sharding: 16 (batch, head) pairs over 8 cores -> each core computes 2 heads
of one batch and returns a partial [N, D] output; host sums 4 partials per
batch.

Per-core dataflow (all layouts transposed, q/m on free dims so softmax'
normalization can be deferred):
  XT = dma-transpose(x)                [D, N]  bf16 (xbar transpose needs 2B)
  QT/KT/VT = W.T @ XT                  [2*HS, N] per head pair (scale folded
                                       into Wq on host)
  S^T[m,q] = KT_h.T @ QT_h             PSUM fp32, per m-chunk of 128
  P^T = exp(S^T)                       ACT, -> SBUF bf16 (no max subtraction:
                                       logits are O(6) by construction)
  O^T[65,q] = [V_h | 1].T @ P^T        PSUM accumulate over m; row 64 = row
                                       sums r[q] (ones-column trick)
  U = O^T -> SBUF; Un = U[0:64] / r    (recip + partition broadcast)
  out[q,:] += Un_h.T @ Wp_h            accumulated over both heads in PSUM
"""

import os
import sys

import numpy as np

sys.path.insert(0, "/opt/trn_rl_repo")

import ml_dtypes
from contextlib import ExitStack

import concourse.bass as bass
import concourse.mybir as mybir
import concourse.tile as tile
from concourse import bacc
from concourse.bass_utils import run_bass_kernel_spmd
from concourse.masks import make_identity

B, N, D, H, HS = 2, 2048, 512, 8, 64
NCORES = 8
BF16 = mybir.dt.bfloat16
FP32 = mybir.dt.float32
nbf16 = ml_dtypes.bfloat16

DC = D // 128  # 4 d-chunks
MC = N // 128  # 16 m-chunks
QH = 2  # q halves
QW = N // QH  # 1024 q per chunk


def build_nc(finalize=True, repeat=1):
    nc = bacc.Bacc()
    xq = nc.dram_tensor("xq", [N, D], BF16, kind="ExternalInput")
    xk = nc.dram_tensor("xk", [N, D], BF16, kind="ExternalInput")
    xv = nc.dram_tensor("xv", [N, D], BF16, kind="ExternalInput")
    wq = nc.dram_tensor("wq", [D, 128], BF16, kind="ExternalInput")
    wk = nc.dram_tensor("wk", [D, 128], BF16, kind="ExternalInput")
    wv = nc.dram_tensor("wv", [D, 128], BF16, kind="ExternalInput")
    wp = nc.dram_tensor("wp", [2 * HS, D], BF16, kind="ExternalInput")
    out = nc.dram_tensor("out", [N, D], FP32, kind="ExternalOutput")

    with tile.TileContext(nc) as tc, ExitStack() as ctx:
        consts = ctx.enter_context(tc.tile_pool(name="consts", bufs=1))
        xt_pool = ctx.enter_context(tc.tile_pool(name="xt", bufs=1))
        proj_pool = ctx.enter_context(tc.tile_pool(name="proj", bufs=1))
        pt_pool = ctx.enter_context(tc.tile_pool(name="pt", bufs=6))
        u_pool = ctx.enter_context(tc.tile_pool(name="u", bufs=4))
        un_pool = ctx.enter_context(tc.tile_pool(name="un", bufs=4))
        rb_pool = ctx.enter_context(tc.tile_pool(name="rb", bufs=2))
        ob_pool = ctx.enter_context(tc.tile_pool(name="ob", bufs=3))
        psA = ctx.enter_context(tc.tile_pool(name="psA", bufs=2, space="PSUM"))
        psO = ctx.enter_context(tc.tile_pool(name="psO", bufs=2, space="PSUM"))

        for _rep in range(repeat):
            ident = consts.tile([128, 128], BF16)
            make_identity(nc, ident[:])

            # weights
            wq_s = consts.tile([128, DC, 128], BF16, tag="wq_s")
            wk_s = consts.tile([128, DC, 128], BF16, tag="wk_s")
            wv_s = consts.tile([128, DC, 128], BF16, tag="wv_s")
            for w_s, w_d in ((wq_s, wq), (wk_s, wk), (wv_s, wv)):
                nc.sync.dma_start(
                    out=w_s[:], in_=w_d.rearrange("(c p) h -> p c h", p=128)
                )
            wp_s = consts.tile([2 * HS, D], BF16, tag="wp_s")
            nc.sync.dma_start(out=wp_s[:], in_=wp[:])

            # Vn: [128, head, mc, 65]; col 64 = ones (rowsum trick)
            vn = consts.tile([128, 2, MC, HS + 1], BF16, tag="vn")
            nc.gpsimd.memset(vn[:, :, :, HS : HS + 1], 1.0)
            # lhsT/rhs must share a base partition; the rowsum row lives at
            # partition HS, so put the ones row there too
            ones_row = consts.tile([HS + 1, HS], BF16, tag="ones_row")
            nc.gpsimd.memset(ones_row[HS : HS + 1, :], 1.0)

            # X transposed: [128, dc, N] per tensor via ONE xbar dma transpose
            # (3D out AP: transposed row d lands at partition d%128, chunk
            # d//128 — same layout as per-chunk transposes, one DMA wait).
            xts = {}
            for name, dram in (("q", xq), ("k", xk), ("v", xv)):
                xts[name] = xt_pool.tile(
                    [128, DC, N], BF16, tag=f"xt_{name}", name=f"xt_{name}"
                )
            # halves DMA'd separately, ordered so the first attention chunk
            # (q half 0 + k half 0) is gated by as little DMA as possible
            for name, half in (
                ("k", 0),
                ("q", 0),
                ("k", 1),
                ("v", 0),
                ("v", 1),
                ("q", 1),
            ):
                dram = {"q": xq, "k": xk, "v": xv}[name]
                nc.sync.dma_start(
                    out=xts[name][:, :, half * QW : (half + 1) * QW],
                    in_=dram[half * QW : (half + 1) * QW, :],
                    transpose=True,
                )

            # projections: [2*HS, N] = sum_dc W[dc].T @ XT[dc]
            wmap = {"q": wq_s, "k": wk_s, "v": wv_s}
            projT = {}
            for name in ("q", "k", "v"):
                projT[name] = proj_pool.tile(
                    [128, N], BF16, tag=f"projT_{name}", name=f"projT_{name}"
                )

            def emit_proj_half(name, half):
                ps = psA.tile([128, QW], FP32, tag="ps", name="ps")
                for sl in range(QW // 512):
                    for dc in range(DC):
                        nc.tensor.matmul(
                            ps[:, sl * 512 : (sl + 1) * 512],
                            wmap[name][:, dc, :],
                            xts[name][
                                :,
                                dc,
                                half * QW + sl * 512 : half * QW + (sl + 1) * 512,
                            ],
                            start=(dc == 0),
                            stop=(dc == DC - 1),
                        )
                nc.vector.tensor_copy(
                    projT[name][:, half * QW : (half + 1) * QW], ps[:]
                )

            def emit_vn_block():
                # V natural: transpose VT2 per m-chunk -> [m, V_h0 | V_h1]
                for mc in range(MC):
                    pst = psA.tile([128, 128], BF16, tag="ps", name="pst")
                    nc.tensor.transpose(
                        pst[:], projT["v"][:, mc * 128 : (mc + 1) * 128], ident[:]
                    )
                    nc.vector.tensor_copy(
                        vn[:, :, mc, 0:HS],
                        pst[:].rearrange("p (b c) -> p b c", b=2),
                    )

            # only q-half-0 and k gate the first attention chunk; the rest
            # is emitted inside the first m-loop to overlap with exp waits
            emit_proj_half("k", 0)
            emit_proj_half("q", 0)
            deferred = [
                lambda: emit_proj_half("k", 1),
                lambda: emit_proj_half("v", 0),
                lambda: emit_proj_half("v", 1),
                emit_vn_block,
                lambda: emit_proj_half("q", 1),
            ]

            # attention + output projection — both heads' m-loops
            # interleaved so the PE stream stays dense (holds HAM warm)
            qt2, kt2 = projT["q"], projT["k"]
            for qh in range(QH):
                un2 = un_pool.tile([128, QW], BF16, tag="un")
                o_ps = {}
                for hh in range(2):
                    o_ps[hh] = psO.tile(
                        [HS + 1, QW], FP32, tag="o", name=f"o_ps{hh}"
                    )

                def pv(hh, j, p_sb):
                    for sl in range(QW // 512):
                        nc.tensor.matmul(
                            o_ps[hh][:, sl * 512 : (sl + 1) * 512],
                            vn[:, hh, j, :],
                            p_sb[:, sl * 512 : (sl + 1) * 512],
                            start=(j == 0),
                            stop=(j == MC - 1),
                        )

                pend = []
                for mc in range(MC):
                    for hh in range(2):
                        hs0 = HS * hh
                        s_ps = psA.tile([128, QW], FP32, tag="ps", name="s_ps")
                        for sl in range(QW // 512):
                            nc.tensor.matmul(
                                s_ps[:, sl * 512 : (sl + 1) * 512],
                                kt2[hs0 : hs0 + HS, mc * 128 : (mc + 1) * 128],
                                qt2[
                                    hs0 : hs0 + HS,
                                    qh * QW + sl * 512 : qh * QW + (sl + 1) * 512,
                                ],
                                start=True,
                                stop=True,
                            )
                        p_sb = pt_pool.tile([128, QW], BF16, tag="p", name="p_sb")
                        nc.scalar.activation(
                            p_sb[:], s_ps[:], mybir.ActivationFunctionType.Exp
                        )
                        if deferred:
                            deferred.pop(0)()
                        pend.append((hh, mc, p_sb))
                        lag = 5 if mc < MC - 1 else 1
                        while len(pend) > lag:
                            pv(*pend.pop(0))
                for e in pend:
                    pv(*e)
                def emit_uchain(hh, o_ps_=None, un2_=None):
                    o_ps_ = o_ps_ if o_ps_ is not None else o_ps
                    un2_ = un2_ if un2_ is not None else un2
                    u = u_pool.tile([HS + 1, QW], BF16, tag="u", name="u")
                    nc.vector.tensor_copy(u[:], o_ps_[hh][:])
                    # broadcast row sums r to 64 partitions via ones.T @ r
                    # (rb reuses the freed o_ps slot)
                    rb_ps = psO.tile([HS, QW], FP32, tag="o", name="rb_ps")
                    for sl in range(QW // 512):
                        nc.tensor.matmul(
                            rb_ps[:, sl * 512 : (sl + 1) * 512],
                            ones_row[HS : HS + 1, :],
                            u[HS : HS + 1, sl * 512 : (sl + 1) * 512],
                            start=True,
                            stop=True,
                        )
                    rb = rb_pool.tile([HS, QW], FP32, tag="rb", name="rb")
                    nc.vector.reciprocal_approx_fast(rb[:], rb_ps[:])
                    nc.vector.tensor_mul(
                        un2_[HS * hh : HS * hh + HS, :], u[0:HS, :], rb[:]
                    )

                if qh < QH - 1:
                    # trickle the normalization chains through the next
                    # m-loop so its S^T stream isn't queued behind them
                    for hh in range(2):
                        deferred.append(
                            lambda hh_=hh, o_=o_ps, u_=un2: emit_uchain(
                                hh_, o_, u_
                            )
                        )
                else:
                    for hh in range(2):
                        emit_uchain(hh)
                # output projection: both heads stacked on 128 partitions —
                # the contraction itself performs the head sum

                def emit_final(qh_, un2_, c):
                    f_ps = psA.tile([128, D], FP32, tag="ps", name="f_ps")
                    nc.tensor.matmul(
                        f_ps[:],
                        un2_[:, c * 128 : (c + 1) * 128],
                        wp_s[:],
                        start=True,
                        stop=True,
                    )
                    ob = ob_pool.tile([128, D], FP32, tag="ob", name="ob")
                    nc.vector.tensor_copy(ob[:], f_ps[:])
                    nc.sync.dma_start(
                        out=out[
                            qh_ * QW + c * 128 : qh_ * QW + (c + 1) * 128, :
                        ],
                        in_=ob[:],
                    )

                if qh < QH - 1:
                    # trickle these through the next m-loop's spare slots
                    for c in range(QW // 128):
                        deferred.append(
                            lambda qh_=qh, un2_=un2, c_=c: emit_final(qh_, un2_, c_)
                        )
                else:
                    for c in range(QW // 128):
                        emit_final(qh, un2, c)
    if finalize:
        nc.finalize()
    return nc


_NC_CACHE = None


def _get_nc():
    global _NC_CACHE
    if _NC_CACHE is None:
        _NC_CACHE = build_nc()
    return _NC_CACHE


def make_in_maps(inputs):
    query = np.asarray(inputs["query"], np.float32)
    key = np.asarray(inputs["key"], np.float32)
    value = np.asarray(inputs["value"], np.float32)
    Wq = np.asarray(inputs["Wq"], np.float32) / np.sqrt(np.float32(HS))
    Wk = np.asarray(inputs["Wk"], np.float32)
    Wv = np.asarray(inputs["Wv"], np.float32)
    Wp = np.asarray(inputs["Wp"], np.float32)

    in_maps = []
    for c in range(NCORES):
        b = c // 4
        h0 = 2 * (c % 4)
        in_maps.append(
            {
                "xq": query[b].astype(nbf16),
                "xk": key[b].astype(nbf16),
                "xv": value[b].astype(nbf16),
                "wq": np.concatenate([Wq[h0], Wq[h0 + 1]], axis=1).astype(nbf16),
                "wk": np.concatenate([Wk[h0], Wk[h0 + 1]], axis=1).astype(nbf16),
                "wv": np.concatenate([Wv[h0], Wv[h0 + 1]], axis=1).astype(nbf16),
                "wp": np.concatenate([Wp[h0], Wp[h0 + 1]], axis=0).astype(nbf16),
            }
        )
    return in_maps


def kernel(query, key, value, Wq, Wk, Wv, Wp):
    in_maps = make_in_maps(
        dict(query=query, key=key, value=value, Wq=Wq, Wk=Wk, Wv=Wv, Wp=Wp)
    )
    nc = _get_nc()
    res = run_bass_kernel_spmd(nc, in_maps, list(range(NCORES)))
    out = np.zeros((B, N, D), np.float32)
    for c in range(NCORES):
        out[c // 4] += np.asarray(res.results[c]["out"], np.float32)
    return out


if __name__ == "__main__":
    d = np.load("/root/problem/work/ref.npz")
    got = kernel(
        d["query"], d["key"], d["value"], d["Wq"], d["Wk"], d["Wv"], d["Wp"]
    )
    exp = d["expected"]
    rel = np.linalg.norm(got - exp) / np.linalg.norm(exp)
    print("Relative error:", rel)



# revision 4
# speedup vs baseline: 1.1107x; 1.1107x over previous
"""Multi-head attention Trainium2 kernel, 8-core SPMD.

Sharding: 16 (batch, head) pairs over 8 cores -> each core computes 2 heads
of one batch and returns a partial [N, D] output (bf16); host sums 4
partials per batch in fp32.

v2 dataflow (all transposed layouts prepared on HOST -- no xbar DMA
transposes, straight contiguous loads only):
  host:  xT[p, c, n] = x[b].T reshaped        [128, DC, N] bf16
  QT/KT = W.T @ xT   per q-quarter slab       [128, N] bf16 (scale folded
                                              into Wq on host)
  Vn    = xT_v.T @ Wv  per m-chunk (natural)  [128 m, 2, 65]; col 64 = ones
                                              (rowsum trick)
  unit (qq, mc): both heads' S via concurrent PE row-tiles (K=64):
    S_h  = KT_h.T @ QT_h -> s2[:, h*512:]     [128 m, 1024] PSUM fp32
    P    = exp(s2)       one ACT op, 1024-free -> p2 bf16
    O_h += [V_h | 1].T @ P_h                  [65, 512] PSUM, accum over mc
  u-chain: r = O[64]; rb = ones.T @ r (bcast); un = O[0:64] * 1/rb
  out[q,:] = un2.T @ Wp (head sum via 128-contraction), bf16 store
"""

import os
import sys

import numpy as np

sys.path.insert(0, "/opt/trn_rl_repo")

import ml_dtypes
from contextlib import ExitStack

import concourse.bass as bass
import concourse.mybir as mybir
import concourse.tile as tile
from concourse import bacc
from concourse.bass_utils import run_bass_kernel_spmd

B, N, D, H, HS = 2, 2048, 512, 8, 64
NCORES = 8
BF16 = mybir.dt.bfloat16
FP32 = mybir.dt.float32
nbf16 = ml_dtypes.bfloat16

DC = D // 128  # 4 d-chunks
MC = N // 128  # 16 m-chunks
QQ = 4  # q quarters
QV = N // QQ  # 512 q per quarter
SLAB = 512  # proj/dma slab width
NSLAB = N // SLAB
PV_LAG = 3  # units between exp and its PV in the PE queue


def build_nc(finalize=True, repeat=1):
    nc = bacc.Bacc()
    xq = nc.dram_tensor("xq", [128, DC, N], BF16, kind="ExternalInput")
    xk = nc.dram_tensor("xk", [128, DC, N], BF16, kind="ExternalInput")
    xv = nc.dram_tensor("xv", [128, DC, N], BF16, kind="ExternalInput")
    wq = nc.dram_tensor("wq", [128, DC, 128], BF16, kind="ExternalInput")
    wk = nc.dram_tensor("wk", [128, DC, 128], BF16, kind="ExternalInput")
    wv = nc.dram_tensor("wv", [128, DC, 128], BF16, kind="ExternalInput")
    wp = nc.dram_tensor("wp", [2 * HS, D], BF16, kind="ExternalInput")
    out = nc.dram_tensor("out", [N, D], BF16, kind="ExternalOutput")

    with tile.TileContext(nc) as tc, ExitStack() as ctx:
        consts = ctx.enter_context(tc.tile_pool(name="consts", bufs=1))
        xt_pool = ctx.enter_context(tc.tile_pool(name="xt", bufs=1))
        kq_pool = ctx.enter_context(tc.tile_pool(name="kq", bufs=1))
        pt_pool = ctx.enter_context(tc.tile_pool(name="pt", bufs=8))
        un_pool = ctx.enter_context(tc.tile_pool(name="un", bufs=2))
        rs_pool = ctx.enter_context(tc.tile_pool(name="rs", bufs=2))
        rb_pool = ctx.enter_context(tc.tile_pool(name="rb", bufs=2))
        ob_pool = ctx.enter_context(tc.tile_pool(name="ob", bufs=3))
        psA = ctx.enter_context(tc.tile_pool(name="psA", bufs=2, space="PSUM"))
        psO = ctx.enter_context(tc.tile_pool(name="psO", bufs=4, space="PSUM"))

        for _rep in range(repeat):
            # constants / weights
            wq_s = consts.tile([128, DC, 128], BF16, tag="wq_s")
            wk_s = consts.tile([128, DC, 128], BF16, tag="wk_s")
            wv_s = consts.tile([128, DC, 128], BF16, tag="wv_s")
            wp_s = consts.tile([2 * HS, D], BF16, tag="wp_s")
            for w_s, w_d in ((wq_s, wq), (wk_s, wk), (wv_s, wv), (wp_s, wp)):
                nc.sync.dma_start(out=w_s[:], in_=w_d[:])

            # Vn: [128 m, mc, head, 65]; col 64 = ones (rowsum trick)
            vn = consts.tile([128, MC, 2, HS + 1], BF16, tag="vn")
            nc.gpsimd.memset(vn[:, :, :, HS : HS + 1], 1.0)
            # rowsum-broadcast ones row lives at partition HS (=64) so the
            # lhsT/rhs base partitions match
            ones_row = consts.tile([HS + 1, HS], BF16, tag="ones_row")
            nc.gpsimd.memset(ones_row[HS : HS + 1, :], 1.0)
            # warm the ACT exp table while DMAs stream
            warm = consts.tile([1, 1], BF16, tag="warm")
            nc.scalar.activation(
                warm[:], ones_row[HS : HS + 1, 0:1],
                mybir.ActivationFunctionType.Exp,
            )

            # X (pre-transposed on host): straight slab loads, ordered so
            # the first attention units are gated by as little DMA as
            # possible: k0, q0, then k/v interleaved, then q rest
            xts = {}
            for name, dram in (("q", xq), ("k", xk), ("v", xv)):
                xts[name] = xt_pool.tile(
                    [128, DC, N], BF16, tag=f"xt_{name}", name=f"xt_{name}"
                )
            dma_order = [
                ("k", 0), ("q", 0), ("k", 1), ("v", 0), ("k", 2), ("v", 1),
                ("k", 3), ("v", 2), ("v", 3), ("q", 1), ("q", 2), ("q", 3),
            ]
            for name, j in dma_order:
                dram = {"q": xq, "k": xk, "v": xv}[name]
                nc.sync.dma_start(
                    out=xts[name][:, :, j * SLAB : (j + 1) * SLAB],
                    in_=dram[:, :, j * SLAB : (j + 1) * SLAB],
                )

            kt2 = kq_pool.tile([128, N], BF16, tag="kt2", name="kt2")
            qt2 = kq_pool.tile([128, N], BF16, tag="qt2", name="qt2")
            dst2 = {"k": kt2, "q": qt2}
            wmap = {"k": wk_s, "q": wq_s}

            def proj_slab(name, j):
                pr = psA.tile([128, SLAB], FP32, tag="ps", name="pr")
                for dc in range(DC):
                    nc.tensor.matmul(
                        pr[:],
                        wmap[name][:, dc, :],
                        xts[name][:, dc, j * SLAB : (j + 1) * SLAB],
                        start=(dc == 0),
                        stop=(dc == DC - 1),
                    )
                nc.vector.tensor_copy(
                    dst2[name][:, j * SLAB : (j + 1) * SLAB], pr[:]
                )

            def vproj_slab(j):
                # V in natural [m, hs2] orientation: 4 m-chunks per slab
                vp = psA.tile([128, 512], FP32, tag="ps", name="vp")
                for m4 in range(4):
                    mc = j * 4 + m4
                    for dc in range(DC):
                        nc.tensor.matmul(
                            vp[:, m4 * 128 : (m4 + 1) * 128],
                            xts["v"][:, dc, mc * 128 : (mc + 1) * 128],
                            wv_s[:, dc, :],
                            start=(dc == 0),
                            stop=(dc == DC - 1),
                        )
                nc.vector.tensor_copy(
                    vn[:, j * 4 : (j + 1) * 4, :, 0:HS],
                    vp[:].rearrange("p (m b c) -> p m b c", m=4, b=2),
                )

            def u_chain(o_t, h, un_t):
                # r = rowsums (row 64 of o); broadcast to 64 partitions via
                # ones.T @ r; un = o[0:64] * 1/rb
                r_sb = rs_pool.tile([HS + 1, QV], BF16, tag="r")
                nc.vector.tensor_copy(r_sb[HS : HS + 1, :], o_t[HS : HS + 1, :])
                rb_ps = psA.tile([HS, QV], FP32, tag="ps", name="rb_ps")
                nc.tensor.matmul(
                    rb_ps[:],
                    ones_row[HS : HS + 1, :],
                    r_sb[HS : HS + 1, :],
                    start=True,
                    stop=True,
                )
                rb = rb_pool.tile([HS, QV], FP32, tag="rb")
                nc.vector.reciprocal_approx_fast(rb[:], rb_ps[:])
                nc.vector.tensor_mul(
                    un_t[HS * h : HS * h + HS, :], o_t[0:HS, :], rb[:]
                )

            def emit_final(qq_, un_t, c):
                f_ps = psA.tile([128, D], FP32, tag="ps", name="f_ps")
                nc.tensor.matmul(
                    f_ps[:],
                    un_t[:, c * 128 : (c + 1) * 128],
                    wp_s[:],
                    start=True,
                    stop=True,
                )
                ob = ob_pool.tile([128, D], BF16, tag="ob", name="ob")
                nc.vector.tensor_copy(ob[:], f_ps[:])
                nc.sync.dma_start(
                    out=out[qq_ * QV + c * 128 : qq_ * QV + (c + 1) * 128, :],
                    in_=ob[:],
                )

            # K slab 0 + Q slab 0 gate the first S; everything else is
            # trickled through the attention stream's deferred slots
            proj_slab("k", 0)
            proj_slab("q", 0)
            deferred = [
                lambda: proj_slab("k", 1),
                lambda: vproj_slab(0),
                lambda: proj_slab("k", 2),
                lambda: vproj_slab(1),
                lambda: proj_slab("k", 3),
                lambda: vproj_slab(2),
                lambda: vproj_slab(3),
                lambda: proj_slab("q", 1),
            ]

            for qq in range(QQ):
                o_ps = [
                    psO.tile([HS + 1, QV], FP32, tag="o", name=f"o{h}")
                    for h in range(2)
                ]
                un2 = un_pool.tile([128, QV], BF16, tag="un")

                def pv(mc, p_sb, o_ps_=o_ps):
                    for h in range(2):
                        nc.tensor.matmul(
                            o_ps_[h][:],
                            vn[:, mc, h, :],
                            p_sb[:, h * QV : (h + 1) * QV],
                            start=(mc == 0),
                            stop=(mc == MC - 1),
                        )

                pend = []
                for mc in range(MC):
                    s2 = psA.tile([128, 1024], FP32, tag="ps", name="s2")
                    for h in range(2):
                        nc.tensor.matmul(
                            s2[:, h * QV : (h + 1) * QV],
                            kt2[h * HS : (h + 1) * HS, mc * 128 : (mc + 1) * 128],
                            qt2[h * HS : (h + 1) * HS, qq * QV : (qq + 1) * QV],
                            start=True,
                            stop=True,
                            tile_position=(h * HS, 0),
                        )
                    p_sb = pt_pool.tile([128, 1024], BF16, tag="p", name="p_sb")
                    nc.scalar.activation(
                        p_sb[:], s2[:], mybir.ActivationFunctionType.Exp
                    )
                    if deferred:
                        deferred.pop(0)()
                    pend.append((mc, p_sb))
                    while len(pend) > PV_LAG:
                        pv(*pend.pop(0))

                def tail(qq_=qq, o_ps_=o_ps, un_t=un2):
                    for h in range(2):
                        u_chain(o_ps_[h], h, un_t)
                    for c in range(QV // 128):
                        emit_final(qq_, un_t, c)

                if qq < QQ - 1:
                    # trickle the prior quarter's trailing PVs,
                    # normalization + output projection through the next
                    # quarter's stream (PVs must precede tail in the queue)
                    for e in pend:
                        deferred.append(lambda e=e, pvf=pv: pvf(*e))
                    pend.clear()
                    deferred.append(lambda t=tail: t())
                    if qq == 0:
                        deferred.append(lambda: proj_slab("q", 2))
                    elif qq == 1:
                        deferred.append(lambda: proj_slab("q", 3))
                else:
                    for e in pend:
                        pv(*e)
                    tail()
    if finalize:
        nc.finalize()
    return nc


_NC_CACHE = None


def _get_nc():
    global _NC_CACHE
    if _NC_CACHE is None:
        _NC_CACHE = build_nc()
    return _NC_CACHE


def _prep_xt(x):
    # [N, D] fp32 -> [128, DC, N] bf16 with xt[p, c, n] = x[n, c*128+p]
    xt = np.ascontiguousarray(
        x.T.reshape(DC, 128, N).transpose(1, 0, 2)
    ).astype(nbf16)
    return xt


def _prep_w(w2):
    # [D, 128] -> [128, DC, 128] with w[p, c, h] = w2[c*128+p, h]
    return np.ascontiguousarray(
        w2.reshape(DC, 128, 128).transpose(1, 0, 2)
    ).astype(nbf16)


def make_in_maps(inputs):
    query = np.asarray(inputs["query"], np.float32)
    key = np.asarray(inputs["key"], np.float32)
    value = np.asarray(inputs["value"], np.float32)
    Wq = np.asarray(inputs["Wq"], np.float32) / np.sqrt(np.float32(HS))
    Wk = np.asarray(inputs["Wk"], np.float32)
    Wv = np.asarray(inputs["Wv"], np.float32)
    Wp = np.asarray(inputs["Wp"], np.float32)

    in_maps = []
    for c in range(NCORES):
        b = c // 4
        h0 = 2 * (c % 4)
        in_maps.append(
            {
                "xq": _prep_xt(query[b]),
                "xk": _prep_xt(key[b]),
                "xv": _prep_xt(value[b]),
                "wq": _prep_w(np.concatenate([Wq[h0], Wq[h0 + 1]], axis=1)),
                "wk": _prep_w(np.concatenate([Wk[h0], Wk[h0 + 1]], axis=1)),
                "wv": _prep_w(np.concatenate([Wv[h0], Wv[h0 + 1]], axis=1)),
                "wp": np.concatenate([Wp[h0], Wp[h0 + 1]], axis=0).astype(nbf16),
            }
        )
    return in_maps


def kernel(query, key, value, Wq, Wk, Wv, Wp):
    in_maps = make_in_maps(
        dict(query=query, key=key, value=value, Wq=Wq, Wk=Wk, Wv=Wv, Wp=Wp)
    )
    nc = _get_nc()
    res = run_bass_kernel_spmd(nc, in_maps, list(range(NCORES)))
    out = np.zeros((B, N, D), np.float32)
    for c in range(NCORES):
        out[c // 4] += np.asarray(res.results[c]["out"], np.float32)
    return out


if __name__ == "__main__":
    d = np.load("/root/problem/work/ref.npz")
    got = kernel(
        d["query"], d["key"], d["value"], d["Wq"], d["Wk"], d["Wv"], d["Wp"]
    )
    exp = d["expected"]
    rel = np.linalg.norm(got - exp) / np.linalg.norm(exp)
    print("Relative error:", rel)


# revision 10
# speedup vs baseline: 1.2386x; 1.1151x over previous
"""Multi-head attention Trainium2 kernel, 8-core SPMD.

Sharding: 16 (batch, head) pairs over 8 cores -> each core computes 2 heads
of one batch and returns a partial [N, D] output (bf16); host sums 4
partials per batch in fp32.

v3 dataflow (all transposed layouts prepared on HOST -- no xbar DMA
transposes, straight contiguous loads only):
  host:  xT[p, c, n] = x[b].T reshaped       [128, DC, N] bf16
  QT/KT = W.T @ xT   per q-slab              [128, N] bf16 (scale folded
                                             into Wq on host)
  Vn    = xT_v.T @ Wv  per m-chunk (natural) [128 m, mc, h, 65] bf16;
                                             col 64 = ones (rowsum trick)
  unit (qq, mc): both heads' S via concurrent PE row-tiles (K=64):
    S_h  = KT_h.T @ QT_h -> s2[:, h*512:]    [128 m, 1024] PSUM fp32
    P    = exp(s2)        one ACT op -> bf16 (softmax here is extremely
                          concentrated; fp8 P/V measured 5-7%% rel err)
    O_h += [V_h | 1].T @ P_h                 [65, 512] PSUM, accum over mc
  u-chain (split into fine steps trickled through the next quarter):
    r = O[64]; rb = ones.T @ r (bcast); un = O[0:64] * 1/rb
  out[q,:] = un2.T @ Wp (head sum via 128-contraction), bf16 store
"""

import os
import sys

import numpy as np

sys.path.insert(0, "/opt/trn_rl_repo")

import ml_dtypes
from contextlib import ExitStack

import concourse.bass as bass
import concourse.mybir as mybir
import concourse.tile as tile
from concourse import bacc
from concourse.bass_utils import run_bass_kernel_spmd

B, N, D, H, HS = 2, 2048, 512, 8, 64
NCORES = 8
BF16 = mybir.dt.bfloat16
FP32 = mybir.dt.float32
FP8 = mybir.dt.float8e4
nbf16 = ml_dtypes.bfloat16
nfp8 = ml_dtypes.float8_e4m3

DC = D // 128  # 4 d-chunks
MC = N // 128  # 16 m-chunks
JP = MC // 2  # 8 m-chunk pairs (fp8 DoubleRow PV)
QQ = 4  # q quarters
QV = N // QQ  # 512 q per quarter
SLAB = 512  # proj/dma slab width
PV_LAG = 3  # units between exp and PV in the PE queue


def build_nc(finalize=True, repeat=1):
    nc = bacc.Bacc()
    xq = nc.dram_tensor("xq", [128, DC, N], BF16, kind="ExternalInput")
    xk = nc.dram_tensor("xk", [128, DC, N], BF16, kind="ExternalInput")
    xv = nc.dram_tensor("xv", [128, DC, N], BF16, kind="ExternalInput")
    wq = nc.dram_tensor("wq", [128, DC, 128], BF16, kind="ExternalInput")
    wk = nc.dram_tensor("wk", [128, DC, 128], BF16, kind="ExternalInput")
    wv = nc.dram_tensor("wv", [128, DC, 128], BF16, kind="ExternalInput")
    wp = nc.dram_tensor("wp", [2 * HS, D], BF16, kind="ExternalInput")
    out = nc.dram_tensor("out", [N, D], BF16, kind="ExternalOutput")

    with tile.TileContext(nc) as tc, ExitStack() as ctx:
        consts = ctx.enter_context(tc.tile_pool(name="consts", bufs=1))
        xt_pool = ctx.enter_context(tc.tile_pool(name="xt", bufs=1))
        kq_pool = ctx.enter_context(tc.tile_pool(name="kq", bufs=1))
        pt_pool = ctx.enter_context(tc.tile_pool(name="pt", bufs=6))
        un_pool = ctx.enter_context(tc.tile_pool(name="un", bufs=2))
        rs_pool = ctx.enter_context(tc.tile_pool(name="rs", bufs=2))
        rb_pool = ctx.enter_context(tc.tile_pool(name="rb", bufs=4))
        ob_pool = ctx.enter_context(tc.tile_pool(name="ob", bufs=3))
        psA = ctx.enter_context(tc.tile_pool(name="psA", bufs=2, space="PSUM"))
        psO = ctx.enter_context(tc.tile_pool(name="psO", bufs=4, space="PSUM"))

        for _rep in range(repeat):
            # constants / weights
            wq_s = consts.tile([128, DC, 128], BF16, tag="wq_s")
            wk_s = consts.tile([128, DC, 128], BF16, tag="wk_s")
            wv_s = consts.tile([128, DC, 128], BF16, tag="wv_s")
            wp_s = consts.tile([2 * HS, D], BF16, tag="wp_s")
            for w_s, w_d in ((wq_s, wq), (wk_s, wk), (wv_s, wv), (wp_s, wp)):
                nc.sync.dma_start(out=w_s[:], in_=w_d[:])

            # Vn: [128 m, mc, head, 65]; col HS = ones (rowsum trick)
            vn = consts.tile([128, MC, 2, HS + 1], BF16, tag="vn")
            nc.gpsimd.memset(vn[:, :, :, HS : HS + 1], 1.0)
            # rowsum-broadcast ones row lives at partition HS (=64) so the
            # lhsT/rhs base partitions match
            ones_row = consts.tile([HS + 1, HS], BF16, tag="ones_row")
            nc.gpsimd.memset(ones_row[HS : HS + 1, :], 1.0)
            # warm the ACT exp table while DMAs stream
            warm = consts.tile([1, 1], BF16, tag="warm")
            nc.scalar.activation(
                warm[:], ones_row[HS : HS + 1, 0:1],
                mybir.ActivationFunctionType.Exp,
            )

            # X (pre-transposed on host): straight slab loads, ordered so
            # the first attention units are gated by as little DMA as
            # possible
            xts = {
                "q": xt_pool.tile([128, DC, N], BF16, tag="xt_q", name="xt_q"),
                "k": xt_pool.tile([128, DC, N], BF16, tag="xt_k", name="xt_k"),
                "v": xt_pool.tile([128, DC, N], BF16, tag="xt_v", name="xt_v"),
            }
            dma_order = [
                ("k", 0), ("q", 0), ("v", 0), ("k", 1), ("v", 1), ("k", 2),
                ("v", 2), ("k", 3), ("v", 3), ("q", 1), ("q", 2), ("q", 3),
            ]
            for name, j in dma_order:
                dram = {"q": xq, "k": xk, "v": xv}[name]
                nc.sync.dma_start(
                    out=xts[name][:, :, j * SLAB : (j + 1) * SLAB],
                    in_=dram[:, :, j * SLAB : (j + 1) * SLAB],
                )

            kt2 = kq_pool.tile([128, N], BF16, tag="kt2", name="kt2")
            qt2 = kq_pool.tile([128, N], BF16, tag="qt2", name="qt2")
            dst2 = {"k": kt2, "q": qt2}
            wmap = {"k": wk_s, "q": wq_s}

            def proj_slab(name, j):
                pr = psA.tile([128, SLAB], FP32, tag="ps", name="pr")
                for dc in range(DC):
                    nc.tensor.matmul(
                        pr[:],
                        wmap[name][:, dc, :],
                        xts[name][:, dc, j * SLAB : (j + 1) * SLAB],
                        start=(dc == 0),
                        stop=(dc == DC - 1),
                    )
                nc.vector.tensor_copy(
                    dst2[name][:, j * SLAB : (j + 1) * SLAB], pr[:]
                )

            def vproj_slab(j):
                # V in natural [m, hs2] orientation: 4 m-chunks per slab,
                # written into the DoubleRow-interleaved fp8 layout
                vp = psA.tile([128, 512], FP32, tag="ps", name="vp")
                for m4 in range(4):
                    mc = j * 4 + m4
                    for dc in range(DC):
                        nc.tensor.matmul(
                            vp[:, m4 * 128 : (m4 + 1) * 128],
                            xts["v"][:, dc, mc * 128 : (mc + 1) * 128],
                            wv_s[:, dc, :],
                            start=(dc == 0),
                            stop=(dc == DC - 1),
                        )
                nc.vector.tensor_copy(
                    vn[:, j * 4 : (j + 1) * 4, :, 0:HS],
                    vp[:].rearrange("p (m h c) -> p m h c", m=4, h=2),
                )

            def emit_final(qq_, un_t, c):
                f_ps = psA.tile([128, D], FP32, tag="ps", name="f_ps")
                nc.tensor.matmul(
                    f_ps[:],
                    un_t[:, c * 128 : (c + 1) * 128],
                    wp_s[:],
                    start=True,
                    stop=True,
                )
                ob = ob_pool.tile([128, D], BF16, tag="ob", name="ob")
                nc.vector.tensor_copy(ob[:], f_ps[:])
                nc.sync.dma_start(
                    out=out[qq_ * QV + c * 128 : qq_ * QV + (c + 1) * 128, :],
                    in_=ob[:],
                )

            def tail_steps(qq_, o_ps_, un_t):
                # r = rowsums (row 64 of o); broadcast to 64 partitions via
                # ones.T @ r; un = o[0:64] * 1/rb.  Split into small steps
                # so each PE op only waits on DVE work from >=1 slot ago.
                r_sb = rs_pool.tile([HS + 1, 2, QV], BF16, tag="r")
                rbs = [None, None]

                def cpy():
                    for h in range(2):
                        nc.vector.tensor_copy(
                            r_sb[HS : HS + 1, h, :], o_ps_[h][HS : HS + 1, :]
                        )

                def rbmm(h):
                    rb_ps = psA.tile([HS, QV], FP32, tag="ps", name="rb_ps")
                    nc.tensor.matmul(
                        rb_ps[:],
                        ones_row[HS : HS + 1, :],
                        r_sb[HS : HS + 1, h, :],
                        start=True,
                        stop=True,
                    )
                    rbs[h] = rb_pool.tile([HS, QV], FP32, tag="rb", name="rb")
                    nc.vector.reciprocal_approx_fast(rbs[h][:], rb_ps[:])

                def mul(h):
                    nc.vector.tensor_mul(
                        un_t[HS * h : HS * h + HS, :],
                        o_ps_[h][0:HS, :],
                        rbs[h][:],
                    )

                return [
                    cpy,
                    lambda: rbmm(0),
                    lambda: rbmm(1),
                    lambda: mul(0),
                    lambda: mul(1),
                    lambda: emit_final(qq_, un_t, 0),
                    lambda: emit_final(qq_, un_t, 1),
                    lambda: emit_final(qq_, un_t, 2),
                    lambda: emit_final(qq_, un_t, 3),
                ]

            # K slab 0 + Q slab 0 gate the first S; everything else is
            # trickled through the attention stream's deferred slots
            proj_slab("k", 0)
            proj_slab("q", 0)
            deferred = [
                lambda: vproj_slab(0),
                lambda: proj_slab("k", 1),
                lambda: vproj_slab(1),
                lambda: proj_slab("k", 2),
                lambda: vproj_slab(2),
                lambda: proj_slab("k", 3),
                lambda: vproj_slab(3),
                lambda: proj_slab("q", 1),
            ]

            for qq in range(QQ):
                o_ps = [
                    psO.tile([HS + 1, QV], FP32, tag="o", name=f"o{h}")
                    for h in range(2)
                ]
                un2 = un_pool.tile([128, QV], BF16, tag="un")

                def pv(mc, p_sb, o_ps_=o_ps):
                    for h in range(2):
                        nc.tensor.matmul(
                            o_ps_[h][:],
                            vn[:, mc, h, :],
                            p_sb[:, h * QV : (h + 1) * QV],
                            start=(mc == 0),
                            stop=(mc == MC - 1),
                        )

                pend = []
                for mc in range(MC):
                    s2 = psA.tile([128, 1024], FP32, tag="ps", name="s2")
                    for h in range(2):
                        nc.tensor.matmul(
                            s2[:, h * QV : (h + 1) * QV],
                            kt2[h * HS : (h + 1) * HS, mc * 128 : (mc + 1) * 128],
                            qt2[h * HS : (h + 1) * HS, qq * QV : (qq + 1) * QV],
                            start=True,
                            stop=True,
                            tile_position=(h * HS, 0),
                        )
                    p_sb = pt_pool.tile([128, 1024], BF16, tag="p", name="p_sb")
                    nc.scalar.activation(
                        p_sb[:], s2[:], mybir.ActivationFunctionType.Exp
                    )
                    if deferred:
                        deferred.pop(0)()
                    pend.append((mc, p_sb))
                    while len(pend) > PV_LAG:
                        pv(*pend.pop(0))

                if qq < QQ - 1:
                    # trickle the prior quarter's trailing PVs,
                    # normalization + output projection through the next
                    # quarter's stream (PVs must precede tail in the queue)
                    for e in pend:
                        deferred.append(lambda e=e, pvf=pv: pvf(*e))
                    pend.clear()
                    deferred.extend(tail_steps(qq, o_ps, un2))
                    if qq == 0:
                        deferred.append(lambda: proj_slab("q", 2))
                    elif qq == 1:
                        deferred.append(lambda: proj_slab("q", 3))
                else:
                    for e in pend:
                        pv(*e)
                    for step in tail_steps(qq, o_ps, un2):
                        step()
    if finalize:
        nc.finalize()
    return nc


_NC_CACHE = None


def _get_nc():
    global _NC_CACHE
    if _NC_CACHE is None:
        _NC_CACHE = build_nc()
    return _NC_CACHE


def _prep_xt(x, dt):
    # [N, D] fp32 -> [128, DC, N] with xt[p, c, n] = x[n, c*128+p]
    return np.ascontiguousarray(
        x.T.reshape(DC, 128, N).transpose(1, 0, 2)
    ).astype(dt)


def _prep_w(w2, dt):
    # [D, 128] -> [128, DC, 128] with w[p, c, h] = w2[c*128+p, h]
    return np.ascontiguousarray(
        w2.reshape(DC, 128, 128).transpose(1, 0, 2)
    ).astype(dt)


def make_in_maps(inputs):
    query = np.asarray(inputs["query"], np.float32)
    key = np.asarray(inputs["key"], np.float32)
    value = np.asarray(inputs["value"], np.float32)
    Wq = np.asarray(inputs["Wq"], np.float32) / np.sqrt(np.float32(HS))
    Wk = np.asarray(inputs["Wk"], np.float32)
    Wv = np.asarray(inputs["Wv"], np.float32)
    Wp = np.asarray(inputs["Wp"], np.float32)

    in_maps = []
    for c in range(NCORES):
        b = c // 4
        h0 = 2 * (c % 4)
        in_maps.append(
            {
                "xq": _prep_xt(query[b], nbf16),
                "xk": _prep_xt(key[b], nbf16),
                "xv": _prep_xt(value[b], nbf16),
                "wq": _prep_w(
                    np.concatenate([Wq[h0], Wq[h0 + 1]], axis=1), nbf16
                ),
                "wk": _prep_w(
                    np.concatenate([Wk[h0], Wk[h0 + 1]], axis=1), nbf16
                ),
                "wv": _prep_w(
                    np.concatenate([Wv[h0], Wv[h0 + 1]], axis=1), nbf16
                ),
                "wp": np.concatenate([Wp[h0], Wp[h0 + 1]], axis=0).astype(nbf16),
            }
        )
    return in_maps


def kernel(query, key, value, Wq, Wk, Wv, Wp):
    in_maps = make_in_maps(
        dict(query=query, key=key, value=value, Wq=Wq, Wk=Wk, Wv=Wv, Wp=Wp)
    )
    nc = _get_nc()
    res = run_bass_kernel_spmd(nc, in_maps, list(range(NCORES)))
    out = np.zeros((B, N, D), np.float32)
    for c in range(NCORES):
        out[c // 4] += np.asarray(res.results[c]["out"], np.float32)
    return out


if __name__ == "__main__":
    d = np.load("/root/problem/work/ref.npz")
    got = kernel(
        d["query"], d["key"], d["value"], d["Wq"], d["Wk"], d["Wv"], d["Wp"]
    )
    exp = d["expected"]
    rel = np.linalg.norm(got - exp) / np.linalg.norm(exp)
    print("Relative error:", rel)
